# revision 29
# baseline (speedup 1.0000x reference)
"""AutoCorrelation (Autoformer-style) sparse attention kernel for 8 trn2 cores.

Math (exact refactoring of the reference):
  mean_corr[b,j] = <qsum @ (wq @ wk.T), k[b,j]> / (H*L),  qsum = sum_i q[b,i]
  top7 delays d_k + softmax weights w_k over mean_corr
  out[b,l]      = (sum_k w_k * values[b,(l+d_k)%L]) @ (wv@wo)

Sharding: core c handles batch b=c//2, output half h=c%2 (rows [h*1024, ...)).
Host does layout/dtype-only prep (slice/transpose/fp16 cast); all FLOPs on
device.  All heavy matmuls run in fp16 (inputs quantized to fp16, fp32 psum
accumulation); verified numerically: top-7 delay sets match fp32 exactly on
the fixed inputs and overall rel err ~7e-4 (tolerance 2e-2).

Compute placement:
  PE : W1=wq@wkT, qsum, uT=W1^T.qsum, scores=uT.kT, W2=wv@wo, mix cols
       [0,640) via weighted-identity psum accumulation, all out matmuls
  ACT: psum->sbuf copies/downcasts, softmax exp (+accumulator sum), mix
       scaled-copy stream for cols [640,896)
  DVE: topk (split max + max_index), small downcasts, weighted identities,
       adds for the ACT stream, stt chain for cols [896,1024), out staging
  Pool: circular vt extension only (walrus rejects gpsimd stt/psum access)

Cost-model notes (TimelineSim v2): matmul cost = out-free-size x cycles/row
(f16=1); the PE clock ramps ~17us before full speed, so W1/qsum double as
ramp fodder and idle gaps are bridged with warmers; DMA is charged on one
exclusive aggregate device (~2.9us per fp16 MB); W1 must NOT be pre-scaled
by 1/(H*L) in fp16 (subnormals flip the fragile batch-3 top-7 margin) - the
scale lives in the softmax's scale/bias instead.
"""

import numpy as np
from contextlib import ExitStack

import concourse.bass as bass
import concourse.bacc as bacc
import concourse.mybir as mybir
import concourse.tile as tile
from concourse.bass_utils import run_bass_kernel_spmd

B, L, D, H = 4, 2048, 512, 8
HALF = L // 2          # 1024 output rows per core
KTOP = 7               # max(1, int(log(2048))) = 7
EXT = L + HALF         # values extended along L for wrap-free dynamic slicing
P = 128
FT = D // P            # 4 feature tiles
NT = L // P            # 16 sequence tiles
F32 = mybir.dt.float32
F16 = mybir.dt.float16
U32 = mybir.dt.uint32
I32 = mybir.dt.int32
AF = mybir.ActivationFunctionType
ALU = mybir.AluOpType
ENG = mybir.EngineType

# ---- mix column-range split (cols of the 1024 output rows) --------------
# strategy: PE scaled-identity psum accumulation / ACT mul + DVE add pipe /
#           DVE stt chain / Pool stt chain.  Ranges on 128 boundaries.
MIX_PE = (0, 640)      # 5 out chunks (psum tiles split 512+128 per fc)
MIX_ACT = (640, 896)   # 2 out chunks
MIX_DVE = (896, 1024)  # 1 out chunk
MIX_POOL = (1024, 1024)  # Pool cannot run scalar_tensor_tensor (walrus)


def _build():
    nc = bacc.Bacc()
    qp_d = nc.dram_tensor("qp", [P, NT, D], F16, kind="ExternalInput")
    ktp_d = nc.dram_tensor("ktp", [P, FT, L], F16, kind="ExternalInput")
    vt_d = nc.dram_tensor("vt", [P, FT, L], F16, kind="ExternalInput")
    wqk_d = nc.dram_tensor("wqk", [P, 2 * FT, D], F16, kind="ExternalInput")
    wvo_d = nc.dram_tensor("wvo", [P, 2 * FT, D], F16, kind="ExternalInput")
    cst16_d = nc.dram_tensor("cst16", [P, P + 2], F16, kind="ExternalInput")
    cstr_d = nc.dram_tensor("cstr", [1, P + 8], F32, kind="ExternalInput")
    out_d = nc.dram_tensor("out", [HALF, D], F32, kind="ExternalOutput")

    with tile.TileContext(nc) as tc, ExitStack() as ctx:
        big = ctx.enter_context(tc.tile_pool(name="big", bufs=1))
        sm = ctx.enter_context(tc.tile_pool(name="sm", bufs=1))
        psA = ctx.enter_context(
            tc.tile_pool(name="psA", bufs=4, space=bass.MemorySpace.PSUM)
        )
        psB = ctx.enter_context(
            tc.tile_pool(name="psB", bufs=4, space=bass.MemorySpace.PSUM)
        )

        # ---- resident input packs; DMAs in priority order ---------------
        wqk = big.tile([P, 2 * FT, D], F16, tag="wqk")
        nc.sync.dma_start(wqk[:], wqk_d[:])
        wqtp = wqk[:, 0:FT, :]
        wktp = wqk[:, FT:2 * FT, :]

        cst16 = sm.tile([P, P + 2], F16, tag="cst16")
        nc.sync.dma_start(cst16[:], cst16_d[:])
        ident16 = cst16[:, 0:P]
        ones16 = cst16[:, P:P + 1]
        cstr = sm.tile([1, P + 8], F32, tag="cstr")
        nc.sync.dma_start(cstr[:], cstr_d[:])
        ones_row = cstr[0:1, 0:P]
        one1 = cstr[0:1, P:P + 1]

        qp = big.tile([P, NT, D], F16, tag="qp")
        nc.sync.dma_start(qp[:, 0:8, :], qp_d[:, 0:8, :])
        nc.sync.dma_start(qp[:, 8:NT, :], qp_d[:, 8:NT, :])

        ktp = big.tile([P, FT, L], F16, tag="ktp")
        nc.sync.dma_start(ktp[:, 0:2, :], ktp_d[:, 0:2, :])
        nc.sync.dma_start(ktp[:, 2:FT, :], ktp_d[:, 2:FT, :])

        wvo = big.tile([P, 2 * FT, D], F16, tag="wvo")
        nc.sync.dma_start(wvo[:], wvo_d[:])
        wvtp = wvo[:, 0:FT, :]
        wop = wvo[:, FT:2 * FT, :]

        vtE = big.tile([P, FT, EXT], F16, tag="vtE")
        nc.sync.dma_start(vtE[:, :, 0:HALF], vt_d[:, :, 0:HALF])
        nc.sync.dma_start(vtE[:, :, HALF:L], vt_d[:, :, HALF:L])

        # ---- small sbuf tiles -------------------------------------------
        w1_16 = big.tile([P, FT, D], F16, tag="w1")
        w2_16 = big.tile([P, FT, D], F16, tag="w2")
        aux = sm.tile([P, 8], F32, tag="aux")
        wbc = aux[:, 0:7]                # broadcast weights [128,7]
        qsumT16 = sm.tile([P, 8], F16, tag="qsT")   # [:,0:4] qsumT, [:,4:8] uT
        uT16 = qsumT16[:, 4:8]
        srow = sm.tile([1, L + 64 + 2 * D], F32, tag="srow")
        qsum_sb = srow[0:1, L + 64:L + 64 + D]
        s_flat = srow[0:1, 0:L]
        vals8 = srow[0:1, L:L + 8]
        ex = srow[0:1, L + 8:L + 15]
        negm = srow[0:1, L + 16:L + 17]
        se = srow[0:1, L + 17:L + 18]
        rse = srow[0:1, L + 18:L + 19]
        w_sb = srow[0:1, L + 19:L + 26]
        idx8 = srow[0:1, L + 32:L + 40].bitcast(U32)
        wI16 = sm.tile([P, KTOP * P], F16, tag="wI")
        acc16 = big.tile([P, FT, HALF], F16, tag="acc16")
        tk16 = big.tile([P, 2, FT, MIX_ACT[1] - MIX_ACT[0]], F16, tag="tk16")
        vmx16 = big.tile([P, FT, MIX_PE[1] - MIX_PE[0]], F16, tag="vmx16")

        # ---- W1 = wq @ wk.T (fp16), scaled by 1/(H*L) at downcast -------
        ps_w1 = [psA.tile([P, D], F32, tag="psa", bufs=4, name=f"ps_w1_{i}") for i in range(FT)]
        for mc in range(FT):
            for ic in range(FT):
                nc.tensor.matmul(
                    ps_w1[ic][:],
                    wqtp[:, mc, ic * P:(ic + 1) * P],
                    wktp[:, mc, :],
                    start=(mc == 0),
                    stop=(mc == FT - 1),
                )
        # keep W1 at natural scale: scaling by 1/(H*L) here would push the
        # fp16 entries into subnormals (catastrophic rounding, flips topk);
        # the 1/(H*L) moves into the softmax scale/bias instead
        for ic in range(FT):
            nc.scalar.copy(w1_16[:, ic, :], ps_w1[ic][:])

        # ---- qsum = ones^T @ q  (psum f32) ------------------------------
        ps_qsum = psA.tile([1, D], F32, tag="psa", bufs=4, name="ps_qsum")
        for t in range(NT):
            nc.tensor.matmul(
                ps_qsum[:], ones16, qp[:, t, :],
                start=(t == 0), stop=(t == NT - 1),
            )
        nc.scalar.copy(qsum_sb, ps_qsum[:])

        # qsumT16 [128,4] via 4 tiny K=1 matmuls + DVE downcasts
        ps_qT = [psA.tile([P, 1], F32, tag="psa", bufs=4, name=f"ps_qT{c}") for c in range(FT)]
        for c in range(FT):
            nc.tensor.matmul(
                ps_qT[c][:], qsum_sb[0:1, c * P:(c + 1) * P], one1,
                start=True, stop=True,
            )
        for c in range(FT):
            nc.vector.tensor_copy(qsumT16[:, c:c + 1], ps_qT[c][:])

        # ---- uT directly: uT[j] = sum_c qsum[c] * W1[c,j] ---------------
        # (avoids the u row + transpose ping-pong: 16 tiny K-contraction
        # matmuls accumulate uT chunks straight into psum)
        ps_uT = [psA.tile([P, 1], F32, tag="psa", bufs=4, name=f"ps_uT{c}") for c in range(FT)]
        for cc in range(FT):
            for jc in range(FT):
                nc.tensor.matmul(
                    ps_uT[jc][:],
                    w1_16[:, cc, jc * P:(jc + 1) * P],
                    qsumT16[:, cc:cc + 1],
                    start=(cc == 0),
                    stop=(cc == FT - 1),
                )
        for c in range(FT):
            nc.vector.tensor_copy(uT16[:, c:c + 1], ps_uT[c][:])

        # ---- scores s[1,2048] = u . k_j  (4 psum banks of 512) ----------
        ps_s = [psB.tile([1, 512], F32, tag="psb", bufs=4, name=f"ps_s{j}") for j in range(FT)]
        # bank-major: bank j finishes after its 4 cc accs, so copies and the
        # max halves pipeline behind the still-running later banks
        for j in range(FT):
            for cc in range(FT):
                nc.tensor.matmul(
                    ps_s[j][:],
                    uT16[:, cc:cc + 1],
                    ktp[:, cc, j * 512:(j + 1) * 512],
                    start=(cc == 0),
                    stop=(cc == FT - 1),
                )

        # circular extension on Pool (idle until the mix starts)
        nc.gpsimd.tensor_copy(vtE[:, :, L:EXT], vtE[:, :, 0:HALF])

        # ---- W2 = wv @ wo (fp16) on PE while DVE runs the topk ----------
        # psA ring (scores own psB); downcasts on ACT
        ps_w2 = [psA.tile([P, D], F32, tag="psa", bufs=4, name=f"ps_w2_{i}") for i in range(FT)]
        for mc in range(FT):
            for ic in range(FT):
                nc.tensor.matmul(
                    ps_w2[ic][:],
                    wvtp[:, mc, ic * P:(ic + 1) * P],
                    wop[:, mc, :],
                    start=(mc == 0),
                    stop=(mc == FT - 1),
                )
        for ic in range(FT):
            nc.scalar.copy(w2_16[:, ic, :], ps_w2[ic][:])

        # post-W2 warmers: bridge the PE gap across the topk so the p-state
        # stays at full clock; ps_out0 is reset by its start=True acc later
        ps_out0 = psA.tile([P, D], F32, tag="psa", bufs=4, name="ps_out0")
        for i in range(12):
            nc.tensor.matmul(ps_out0[:, 0:P], ident16, ident16,
                             start=True, stop=True)

        for j in range(FT):
            dst = s_flat[0:1, j * 512:(j + 1) * 512]
            if j == 1:
                nc.vector.tensor_copy(dst, ps_s[j][:])
            else:
                nc.scalar.copy(dst, ps_s[j][:])

        # ---- top-8 + softmax over first 7 -------------------------------
        # max in two pipelined halves (each starts as soon as its two score
        # banks are copied), merged by an 8+8 -> top8 pass
        v8a = srow[0:1, L + 40:L + 48]
        v8b = srow[0:1, L + 48:L + 56]
        nc.vector.max(v8a, s_flat[0:1, 0:1024])
        nc.vector.max(v8b, s_flat[0:1, 1024:2048])
        nc.vector.max(vals8, srow[0:1, L + 40:L + 56])
        nc.vector.tensor_scalar_mul(negm, vals8[0:1, 0:1], -1.0 / (H * L))
        nc.vector.max_index(idx8, vals8, s_flat)
        # Exp computes its own sum via the ACT accumulator (one op less on
        # the DVE, which is busy with max_index)
        nc.scalar.activation(
            ex, vals8[0:1, 0:KTOP], AF.Exp, bias=negm, scale=1.0 / (H * L),
            accum_out=se,
        )
        nc.vector.reciprocal(rse, se)
        nc.vector.tensor_scalar_mul(w_sb, ex, rse)

        # broadcast weights along partitions -> wbc [128,7] f32
        ps_wbc = psB.tile([P, KTOP], F32, tag="psb", bufs=4, name="ps_wbc")
        nc.tensor.matmul(ps_wbc[:], ones_row, w_sb, start=True, stop=True)
        nc.scalar.copy(wbc, ps_wbc[:])


        # ---- delays into engine registers (right after topk; engine
        # queues are clean here so the cross-engine load doesn't stall) ----
        _, dks = nc.values_load_multi_w_load_instructions(
            idx8[0:1, 0:KTOP].bitcast(I32),
            engines=(ENG.PE, ENG.DVE, ENG.Activation),
            min_val=0,
            max_val=L - 1,
            skip_runtime_bounds_check=True,
        )

        # weighted identities for the PE mix stream (DVE, JIT per k)
        for kk in range(KTOP):
            nc.vector.tensor_scalar_mul(
                wI16[:, kk * P:(kk + 1) * P], ident16, wbc[:, kk:kk + 1]
            )



        # ---- mix: engine streams over column ranges ---------------------
        # PE: psum accumulation with weighted identities, 512-col groups
        a0, a1 = MIX_PE
        groups = []
        g = a0
        while g < a1:
            gw = min(512, a1 - g)
            groups.append((g, gw))
            g += gw
        for gi, (gb, gw) in enumerate(groups):
            for fc in range(FT):
                ps_mx = psB.tile([P, gw], F32, tag="psb", bufs=4,
                                 name=f"ps_mx{gi}_{fc}")
                for kk in range(KTOP):
                    nc.tensor.matmul(
                        ps_mx[:],
                        wI16[:, kk * P:(kk + 1) * P],
                        vtE[:, fc, bass.ds(dks[kk] + gb, gw)],
                        start=(kk == 0),
                        stop=(kk == KTOP - 1),
                    )
                # downcast as soon as this chunk's accumulation finishes;
                # alternate ACT/DVE so neither serializes the PE stream
                dst = vmx16[:, fc, gb - a0:gb - a0 + gw]
                if fc % 2 == 0:
                    nc.scalar.copy(dst, ps_mx[:])
                else:
                    nc.vector.tensor_copy(dst, ps_mx[:])

        # ACT mul stream + DVE add chain
        b0, b1 = MIX_ACT
        nb = b1 - b0
        accA = acc16[:, :, b0:b1]
        nc.scalar.mul(accA[:], vtE[:, :, bass.ds(dks[0] + b0, nb)], wbc[:, 0:1])
        for kk in range(1, KTOP):
            tkb = tk16[:, kk % 2]
            nc.scalar.mul(
                tkb[:], vtE[:, :, bass.ds(dks[kk] + b0, nb)], wbc[:, kk:kk + 1]
            )
            nc.vector.tensor_tensor(accA[:], tkb[:], accA[:], ALU.add)

        # DVE stt chain (disabled when the range is empty)
        c0, c1 = MIX_DVE
        if c1 > c0:
            ncd = c1 - c0
            accD = acc16[:, :, c0:c1]
            nc.vector.tensor_scalar_mul(
                accD[:], vtE[:, :, bass.ds(dks[0] + c0, ncd)], wbc[:, 0:1]
            )
            for kk in range(1, KTOP):
                nc.vector.scalar_tensor_tensor(
                    accD[:],
                    vtE[:, :, bass.ds(dks[kk] + c0, ncd)],
                    wbc[:, kk:kk + 1],
                    accD[:],
                    ALU.mult,
                    ALU.add,
                )


        # ---- out rows: out[l,:] = sum_f vmixT[f,l] * W2[f,:] ------------
        # psum -> sbuf staging (DMA cannot read PSUM); copies alternate
        # ACT/DVE; one DMA per pair of chunks.
        NH = HALF // P
        ostg = big.tile([P, NH, D], F32, tag="ostg")
        stage_eng = [0, 1, 0, 1, 0, 1, 0, 1]
        for lc in range(NH):
            lo = lc * P
            if lc == 0:
                ps_out = ps_out0
            else:
                ps_out = psA.tile([P, D], F32, tag="psa", bufs=4, name=f"ps_out{lc}")
            for ft in range(FT):
                if a0 <= lo < a1:
                    src = vmx16[:, ft, lo - a0:lo - a0 + P]
                else:
                    src = acc16[:, ft, lo:lo + P]
                nc.tensor.matmul(
                    ps_out[:], src, w2_16[:, ft, :],
                    start=(ft == 0), stop=(ft == FT - 1),
                )
            se_ = stage_eng[lc]
            if se_ == 0:
                nc.scalar.copy(ostg[:, lc, :], ps_out[:])
            elif se_ == 1:
                nc.vector.tensor_copy(ostg[:, lc, :], ps_out[:])
            else:
                nc.gpsimd.tensor_copy(ostg[:, lc, :], ps_out[:])
            nc.sync.dma_start(
                out_d.rearrange("(t p) c -> p t c", p=P)[:, lc:lc + 1, :],
                ostg[:, lc:lc + 1, :],
            )

    return nc


_NC = None
TRACE = False
_LAST_RESULTS = None


def _get_nc():
    global _NC
    if _NC is None:
        _NC = _build()
        _NC.finalize()
    return _NC


def _prep_consts():
    cst16 = np.zeros((P, P + 2), np.float16)
    cst16[:, 0:P] = np.eye(P, dtype=np.float16)
    cst16[:, P] = 1.0
    cstr = np.zeros((1, P + 8), np.float32)
    cstr[0, 0:P] = 1.0
    cstr[0, P] = 1.0
    return cst16, cstr


def kernel(queries, keys, values, wq, wk, wv, wo):
    nc = _get_nc()
    cst16, cstr = _prep_consts()
    f16 = np.float16

    def pack(m):
        # [512, 512] -> [128, 4, 512] with row index m = mc*128 + p
        return np.ascontiguousarray(
            m.reshape(FT, P, D).transpose(1, 0, 2).astype(f16)
        )

    wqk = np.ascontiguousarray(
        np.concatenate([pack(np.asarray(wq).T), pack(np.asarray(wk).T)], axis=1)
    )
    wvo = np.ascontiguousarray(
        np.concatenate([pack(np.asarray(wv).T), pack(np.asarray(wo))], axis=1)
    )

    in_maps = []
    for c in range(8):
        b, h = divmod(c, 2)
        qp = np.ascontiguousarray(
            queries[b].reshape(NT, P, D).transpose(1, 0, 2).astype(f16)
        )
        ktp = np.ascontiguousarray(
            keys[b].T.reshape(FT, P, L).transpose(1, 0, 2).astype(f16)
        )
        vrot = np.roll(values[b], -h * HALF, axis=0)
        vt = np.ascontiguousarray(
            vrot.T.reshape(FT, P, L).transpose(1, 0, 2).astype(f16)
        )
        in_maps.append(
            {
                "qp": qp,
                "ktp": ktp,
                "vt": vt,
                "wqk": wqk,
                "wvo": wvo,
                "cst16": cst16,
                "cstr": cstr,
            }
        )
    global _LAST_RESULTS
    res = run_bass_kernel_spmd(nc, in_maps, list(range(8)), trace=TRACE)
    _LAST_RESULTS = res
    out = np.empty((B, L, D), np.float32)
    for c in range(8):
        b, h = divmod(c, 2)
        out[b, h * HALF:(h + 1) * HALF] = res.results[c]["out"]
    return out


# revision 30
# speedup vs baseline: 1.0543x; 1.0543x over previous
"""AutoCorrelation (Autoformer-style) sparse attention kernel for 8 trn2 cores.

Math (exact refactoring of the reference):
  mean_corr[b,j] = <qsum @ (wq @ wk.T), k[b,j]> / (H*L),  qsum = sum_i q[b,i]
  top7 delays d_k + softmax weights w_k over mean_corr
  out[b,l]      = (sum_k w_k * values[b,(l+d_k)%L]) @ (wv@wo)

Sharding: core c handles batch b=c//2, output half h=c%2 (rows [h*1024, ...)).
Host does layout/dtype-only prep (slice/transpose/fp16 cast); all FLOPs on
device.  All heavy matmuls run in fp16 (inputs quantized to fp16, fp32 psum
accumulation); verified numerically: top-7 delay sets match fp32 exactly on
the fixed inputs and overall rel err ~7e-4 (tolerance 2e-2).

Compute placement:
  PE : W1=wq@wkT, qsum, uT=W1^T.qsum, scores=uT.kT, W2=wv@wo, mix cols
       [0,640) via weighted-identity psum accumulation, all out matmuls
  ACT: psum->sbuf copies/downcasts, softmax exp (+accumulator sum), mix
       scaled-copy stream for cols [640,896)
  DVE: topk (split max + max_index), small downcasts, weighted identities,
       adds for the ACT stream, stt chain for cols [896,1024), out staging
  Pool: circular vt extension only (walrus rejects gpsimd stt/psum access)

Cost-model notes (TimelineSim v2): matmul cost = out-free-size x cycles/row
(f16=1); the PE clock ramps ~17us before full speed, so W1/qsum double as
ramp fodder and idle gaps are bridged with warmers; DMA is charged on one
exclusive aggregate device (~2.9us per fp16 MB); W1 must NOT be pre-scaled
by 1/(H*L) in fp16 (subnormals flip the fragile batch-3 top-7 margin) - the
scale lives in the softmax's scale/bias instead.
"""

import numpy as np
from contextlib import ExitStack

import concourse.bass as bass
import concourse.bacc as bacc
import concourse.mybir as mybir
import concourse.tile as tile
from concourse.bass_utils import run_bass_kernel_spmd

B, L, D, H = 4, 2048, 512, 8
HALF = L // 2          # 1024 output rows per core
KTOP = 7               # max(1, int(log(2048))) = 7
EXT = L + HALF         # values extended along L for wrap-free dynamic slicing
P = 128
FT = D // P            # 4 feature tiles
NT = L // P            # 16 sequence tiles
F32 = mybir.dt.float32
F16 = mybir.dt.float16
U32 = mybir.dt.uint32
I32 = mybir.dt.int32
AF = mybir.ActivationFunctionType
ALU = mybir.AluOpType
ENG = mybir.EngineType

# ---- mix column-range split (cols of the 1024 output rows) --------------
# strategy: PE scaled-identity psum accumulation / ACT mul + DVE add pipe /
#           DVE stt chain / Pool stt chain.  Ranges on 128 boundaries.
MIX_PE = (0, 640)      # psum tiles split 512+128 per fc
MIX_ACT = (640, 960)   # ACT mul stream + DVE adds
MIX_DVE = (960, 1024)  # DVE stt chain (ranges need not align to out chunks)
MIX_POOL = (1024, 1024)  # Pool cannot run scalar_tensor_tensor (walrus)


def _build():
    nc = bacc.Bacc()
    qp_d = nc.dram_tensor("qp", [P, NT, D], F16, kind="ExternalInput")
    ktp_d = nc.dram_tensor("ktp", [P, FT, L], F16, kind="ExternalInput")
    vt_d = nc.dram_tensor("vt", [P, FT, L], F16, kind="ExternalInput")
    wqk_d = nc.dram_tensor("wqk", [P, FT, 2, D], F16, kind="ExternalInput")
    wvo_d = nc.dram_tensor("wvo", [P, 2 * FT, D], F16, kind="ExternalInput")
    cst16_d = nc.dram_tensor("cst16", [P, P + 2], F16, kind="ExternalInput")
    cstr_d = nc.dram_tensor("cstr", [1, P + 8], F32, kind="ExternalInput")
    out_d = nc.dram_tensor("out", [HALF, D], F32, kind="ExternalOutput")

    with tile.TileContext(nc) as tc, ExitStack() as ctx:
        big = ctx.enter_context(tc.tile_pool(name="big", bufs=1))
        sm = ctx.enter_context(tc.tile_pool(name="sm", bufs=1))
        psA = ctx.enter_context(
            tc.tile_pool(name="psA", bufs=4, space=bass.MemorySpace.PSUM)
        )
        psB = ctx.enter_context(
            tc.tile_pool(name="psB", bufs=4, space=bass.MemorySpace.PSUM)
        )

        # ---- resident input packs; DMAs in priority order ---------------
        # wq/wk packs interleaved by contraction chunk: W1's first accs can
        # start as soon as the first half lands (earlier PE p-state ramp)
        wqk = big.tile([P, FT, 2, D], F16, tag="wqk")
        nc.sync.dma_start(wqk[:, 0:2], wqk_d[:, 0:2])
        nc.sync.dma_start(wqk[:, 2:FT], wqk_d[:, 2:FT])

        cst16 = sm.tile([P, P + 2], F16, tag="cst16")
        nc.sync.dma_start(cst16[:], cst16_d[:])
        ident16 = cst16[:, 0:P]
        ones16 = cst16[:, P:P + 1]
        cstr = sm.tile([1, P + 8], F32, tag="cstr")
        nc.sync.dma_start(cstr[:], cstr_d[:])
        ones_row = cstr[0:1, 0:P]
        one1 = cstr[0:1, P:P + 1]

        qp = big.tile([P, NT, D], F16, tag="qp")
        nc.sync.dma_start(qp[:, 0:8, :], qp_d[:, 0:8, :])
        nc.sync.dma_start(qp[:, 8:NT, :], qp_d[:, 8:NT, :])

        ktp = big.tile([P, FT, L], F16, tag="ktp")
        nc.sync.dma_start(ktp[:, 0:2, :], ktp_d[:, 0:2, :])
        nc.sync.dma_start(ktp[:, 2:FT, :], ktp_d[:, 2:FT, :])

        wvo = big.tile([P, 2 * FT, D], F16, tag="wvo")
        nc.sync.dma_start(wvo[:], wvo_d[:])
        wvtp = wvo[:, 0:FT, :]
        wop = wvo[:, FT:2 * FT, :]

        vtE = big.tile([P, FT, EXT], F16, tag="vtE")
        nc.sync.dma_start(vtE[:, :, 0:HALF], vt_d[:, :, 0:HALF])
        nc.sync.dma_start(vtE[:, :, HALF:L], vt_d[:, :, HALF:L])

        # ---- small sbuf tiles -------------------------------------------
        w1_16 = big.tile([P, FT, D], F16, tag="w1")
        w2_16 = big.tile([P, FT, D], F16, tag="w2")
        aux = sm.tile([P, 8], F32, tag="aux")
        wbc = aux[:, 0:7]                # broadcast weights [128,7]
        qsumT16 = sm.tile([P, 8], F16, tag="qsT")   # [:,0:4] qsumT, [:,4:8] uT
        uT16 = qsumT16[:, 4:8]
        srow = sm.tile([1, L + 64 + 2 * D], F32, tag="srow")
        qsum_sb = srow[0:1, L + 64:L + 64 + D]
        s_flat = srow[0:1, 0:L]
        vals8 = srow[0:1, L:L + 8]
        ex = srow[0:1, L + 8:L + 15]
        negm = srow[0:1, L + 16:L + 17]
        se = srow[0:1, L + 17:L + 18]
        rse = srow[0:1, L + 18:L + 19]
        w_sb = srow[0:1, L + 19:L + 26]
        idx8 = srow[0:1, L + 32:L + 40].bitcast(U32)
        wI16 = sm.tile([P, KTOP * P], F16, tag="wI")
        acc16 = big.tile([P, FT, HALF], F16, tag="acc16")
        tk16 = big.tile([P, 2, FT, MIX_ACT[1] - MIX_ACT[0]], F16, tag="tk16")
        vmx16 = big.tile([P, FT, MIX_PE[1] - MIX_PE[0]], F16, tag="vmx16")

        # ---- W1 = wq @ wk.T (fp16), scaled by 1/(H*L) at downcast -------
        ps_w1 = [psA.tile([P, D], F32, tag="psa", bufs=4, name=f"ps_w1_{i}") for i in range(FT)]
        for mc in range(FT):
            for ic in range(FT):
                nc.tensor.matmul(
                    ps_w1[ic][:],
                    wqk[:, mc, 0, ic * P:(ic + 1) * P],
                    wqk[:, mc, 1, :],
                    start=(mc == 0),
                    stop=(mc == FT - 1),
                )
        # keep W1 at natural scale: scaling by 1/(H*L) here would push the
        # fp16 entries into subnormals (catastrophic rounding, flips topk);
        # the 1/(H*L) moves into the softmax scale/bias instead
        for ic in range(FT):
            nc.scalar.copy(w1_16[:, ic, :], ps_w1[ic][:])

        # ---- qsum = ones^T @ q  (psum f32) ------------------------------
        ps_qsum = psA.tile([1, D], F32, tag="psa", bufs=4, name="ps_qsum")
        for t in range(NT):
            nc.tensor.matmul(
                ps_qsum[:], ones16, qp[:, t, :],
                start=(t == 0), stop=(t == NT - 1),
            )
        nc.scalar.copy(qsum_sb, ps_qsum[:])

        # qsumT16 [128,4] via 4 tiny K=1 matmuls + DVE downcasts
        ps_qT = [psA.tile([P, 1], F32, tag="psa", bufs=4, name=f"ps_qT{c}") for c in range(FT)]
        for c in range(FT):
            nc.tensor.matmul(
                ps_qT[c][:], qsum_sb[0:1, c * P:(c + 1) * P], one1,
                start=True, stop=True,
            )
        for c in range(FT):
            nc.vector.tensor_copy(qsumT16[:, c:c + 1], ps_qT[c][:])

        # ---- uT directly: uT[j] = sum_c qsum[c] * W1[c,j] ---------------
        # (avoids the u row + transpose ping-pong: 16 tiny K-contraction
        # matmuls accumulate uT chunks straight into psum)
        ps_uT = [psA.tile([P, 1], F32, tag="psa", bufs=4, name=f"ps_uT{c}") for c in range(FT)]
        for cc in range(FT):
            for jc in range(FT):
                nc.tensor.matmul(
                    ps_uT[jc][:],
                    w1_16[:, cc, jc * P:(jc + 1) * P],
                    qsumT16[:, cc:cc + 1],
                    start=(cc == 0),
                    stop=(cc == FT - 1),
                )
        for c in range(FT):
            nc.vector.tensor_copy(uT16[:, c:c + 1], ps_uT[c][:])

        # ---- scores s[1,2048] = u . k_j  (4 psum banks of 512) ----------
        ps_s = [psB.tile([1, 512], F32, tag="psb", bufs=4, name=f"ps_s{j}") for j in range(FT)]
        # bank-major: bank j finishes after its 4 cc accs, so copies and the
        # max halves pipeline behind the still-running later banks
        for j in range(FT):
            for cc in range(FT):
                nc.tensor.matmul(
                    ps_s[j][:],
                    uT16[:, cc:cc + 1],
                    ktp[:, cc, j * 512:(j + 1) * 512],
                    start=(cc == 0),
                    stop=(cc == FT - 1),
                )

        # circular extension on Pool (idle until the mix starts)
        nc.gpsimd.tensor_copy(vtE[:, :, L:EXT], vtE[:, :, 0:HALF])

        # ---- W2 = wv @ wo (fp16) on PE while DVE runs the topk ----------
        # psA ring (scores own psB); downcasts on ACT
        ps_w2 = [psA.tile([P, D], F32, tag="psa", bufs=4, name=f"ps_w2_{i}") for i in range(FT)]
        for mc in range(FT):
            for ic in range(FT):
                nc.tensor.matmul(
                    ps_w2[ic][:],
                    wvtp[:, mc, ic * P:(ic + 1) * P],
                    wop[:, mc, :],
                    start=(mc == 0),
                    stop=(mc == FT - 1),
                )
        for ic in range(FT):
            nc.scalar.copy(w2_16[:, ic, :], ps_w2[ic][:])

        # post-W2 warmers: bridge the PE gap across the topk so the p-state
        # stays at full clock; ps_out0 is reset by its start=True acc later
        ps_out0 = psA.tile([P, D], F32, tag="psa", bufs=4, name="ps_out0")
        for i in range(12):
            nc.tensor.matmul(ps_out0[:, 0:P], ident16, ident16,
                             start=True, stop=True)

        for j in range(FT):
            dst = s_flat[0:1, j * 512:(j + 1) * 512]
            if j == 1:
                nc.vector.tensor_copy(dst, ps_s[j][:])
            else:
                nc.scalar.copy(dst, ps_s[j][:])

        # ---- top-8 + softmax over first 7 -------------------------------
        # max in two pipelined halves (each starts as soon as its two score
        # banks are copied), merged by an 8+8 -> top8 pass
        v8a = srow[0:1, L + 40:L + 48]
        v8b = srow[0:1, L + 48:L + 56]
        nc.vector.max(v8a, s_flat[0:1, 0:1024])
        nc.vector.max(v8b, s_flat[0:1, 1024:2048])
        nc.vector.max(vals8, srow[0:1, L + 40:L + 56])
        nc.vector.tensor_scalar_mul(negm, vals8[0:1, 0:1], -1.0 / (H * L))
        nc.vector.max_index(idx8, vals8, s_flat)
        # Exp computes its own sum via the ACT accumulator (one op less on
        # the DVE, which is busy with max_index)
        nc.scalar.activation(
            ex, vals8[0:1, 0:KTOP], AF.Exp, bias=negm, scale=1.0 / (H * L),
            accum_out=se,
        )
        nc.vector.reciprocal(rse, se)
        nc.vector.tensor_scalar_mul(w_sb, ex, rse)

        # broadcast weights along partitions -> wbc [128,7] f32
        ps_wbc = psB.tile([P, KTOP], F32, tag="psb", bufs=4, name="ps_wbc")
        nc.tensor.matmul(ps_wbc[:], ones_row, w_sb, start=True, stop=True)
        nc.scalar.copy(wbc, ps_wbc[:])


        # ---- delays into engine registers (right after topk; engine
        # queues are clean here so the cross-engine load doesn't stall) ----
        _, dks = nc.values_load_multi_w_load_instructions(
            idx8[0:1, 0:KTOP].bitcast(I32),
            engines=(ENG.PE, ENG.DVE, ENG.Activation),
            min_val=0,
            max_val=L - 1,
            skip_runtime_bounds_check=True,
        )

        # weighted identities for the PE mix stream (DVE, JIT per k)
        for kk in range(KTOP):
            nc.vector.tensor_scalar_mul(
                wI16[:, kk * P:(kk + 1) * P], ident16, wbc[:, kk:kk + 1]
            )



        # ---- mix: engine streams over column ranges ---------------------
        # PE: psum accumulation with weighted identities, 512-col groups
        a0, a1 = MIX_PE
        groups = []
        g = a0
        while g < a1:
            gw = min(512, a1 - g)
            groups.append((g, gw))
            g += gw
        for gi, (gb, gw) in enumerate(groups):
            for fc in range(FT):
                ps_mx = psB.tile([P, gw], F32, tag="psb", bufs=4,
                                 name=f"ps_mx{gi}_{fc}")
                for kk in range(KTOP):
                    nc.tensor.matmul(
                        ps_mx[:],
                        wI16[:, kk * P:(kk + 1) * P],
                        vtE[:, fc, bass.ds(dks[kk] + gb, gw)],
                        start=(kk == 0),
                        stop=(kk == KTOP - 1),
                    )
                # downcast as soon as this chunk's accumulation finishes;
                # alternate ACT/DVE so neither serializes the PE stream
                dst = vmx16[:, fc, gb - a0:gb - a0 + gw]
                if fc % 2 == 0:
                    nc.scalar.copy(dst, ps_mx[:])
                else:
                    nc.vector.tensor_copy(dst, ps_mx[:])

        # ACT mul stream + DVE add chain
        b0, b1 = MIX_ACT
        nb = b1 - b0
        accA = acc16[:, :, b0:b1]
        nc.scalar.mul(accA[:], vtE[:, :, bass.ds(dks[0] + b0, nb)], wbc[:, 0:1])
        for kk in range(1, KTOP):
            tkb = tk16[:, kk % 2]
            nc.scalar.mul(
                tkb[:], vtE[:, :, bass.ds(dks[kk] + b0, nb)], wbc[:, kk:kk + 1]
            )
            nc.vector.tensor_tensor(accA[:], tkb[:], accA[:], ALU.add)

        # DVE stt chain (disabled when the range is empty)
        c0, c1 = MIX_DVE
        if c1 > c0:
            ncd = c1 - c0
            accD = acc16[:, :, c0:c1]
            nc.vector.tensor_scalar_mul(
                accD[:], vtE[:, :, bass.ds(dks[0] + c0, ncd)], wbc[:, 0:1]
            )
            for kk in range(1, KTOP):
                nc.vector.scalar_tensor_tensor(
                    accD[:],
                    vtE[:, :, bass.ds(dks[kk] + c0, ncd)],
                    wbc[:, kk:kk + 1],
                    accD[:],
                    ALU.mult,
                    ALU.add,
                )


        # ---- out rows: out[l,:] = sum_f vmixT[f,l] * W2[f,:] ------------
        # psum -> sbuf staging (DMA cannot read PSUM); copies alternate
        # ACT/DVE; one DMA per pair of chunks.
        NH = HALF // P
        ostg = big.tile([P, NH, D], F32, tag="ostg")
        stage_eng = [0, 1, 0, 1, 0, 1, 0, 1]
        for lc in range(NH):
            lo = lc * P
            if lc == 0:
                ps_out = ps_out0
            else:
                ps_out = psA.tile([P, D], F32, tag="psa", bufs=4, name=f"ps_out{lc}")
            for ft in range(FT):
                if a0 <= lo < a1:
                    src = vmx16[:, ft, lo - a0:lo - a0 + P]
                else:
                    src = acc16[:, ft, lo:lo + P]
                nc.tensor.matmul(
                    ps_out[:], src, w2_16[:, ft, :],
                    start=(ft == 0), stop=(ft == FT - 1),
                )
            se_ = stage_eng[lc]
            if se_ == 0:
                nc.scalar.copy(ostg[:, lc, :], ps_out[:])
            elif se_ == 1:
                nc.vector.tensor_copy(ostg[:, lc, :], ps_out[:])
            else:
                nc.gpsimd.tensor_copy(ostg[:, lc, :], ps_out[:])
            nc.sync.dma_start(
                out_d.rearrange("(t p) c -> p t c", p=P)[:, lc:lc + 1, :],
                ostg[:, lc:lc + 1, :],
            )

    return nc


_NC = None
TRACE = False
_LAST_RESULTS = None


def _get_nc():
    global _NC
    if _NC is None:
        _NC = _build()
        _NC.finalize()
    return _NC


def _prep_consts():
    cst16 = np.zeros((P, P + 2), np.float16)
    cst16[:, 0:P] = np.eye(P, dtype=np.float16)
    cst16[:, P] = 1.0
    cstr = np.zeros((1, P + 8), np.float32)
    cstr[0, 0:P] = 1.0
    cstr[0, P] = 1.0
    return cst16, cstr


def kernel(queries, keys, values, wq, wk, wv, wo):
    nc = _get_nc()
    cst16, cstr = _prep_consts()
    f16 = np.float16

    def pack(m):
        # [512, 512] -> [128, 4, 512] with row index m = mc*128 + p
        return np.ascontiguousarray(
            m.reshape(FT, P, D).transpose(1, 0, 2).astype(f16)
        )

    wqk = np.ascontiguousarray(
        np.stack([pack(np.asarray(wq).T), pack(np.asarray(wk).T)], axis=2)
    )
    wvo = np.ascontiguousarray(
        np.concatenate([pack(np.asarray(wv).T), pack(np.asarray(wo))], axis=1)
    )

    in_maps = []
    for c in range(8):
        b, h = divmod(c, 2)
        qp = np.ascontiguousarray(
            queries[b].reshape(NT, P, D).transpose(1, 0, 2).astype(f16)
        )
        ktp = np.ascontiguousarray(
            keys[b].T.reshape(FT, P, L).transpose(1, 0, 2).astype(f16)
        )
        vrot = np.roll(values[b], -h * HALF, axis=0)
        vt = np.ascontiguousarray(
            vrot.T.reshape(FT, P, L).transpose(1, 0, 2).astype(f16)
        )
        in_maps.append(
            {
                "qp": qp,
                "ktp": ktp,
                "vt": vt,
                "wqk": wqk,
                "wvo": wvo,
                "cst16": cst16,
                "cstr": cstr,
            }
        )
    global _LAST_RESULTS
    res = run_bass_kernel_spmd(nc, in_maps, list(range(8)), trace=TRACE)
    _LAST_RESULTS = res
    out = np.empty((B, L, D), np.float32)
    for c in range(8):
        b, h = divmod(c, 2)
        out[b, h * HALF:(h + 1) * HALF] = res.results[c]["out"]
    return out


# revision 47
# speedup vs baseline: 1.1317x; 1.0734x over previous
"""AutoCorrelation (Autoformer-style) sparse attention kernel for 8 trn2 cores.

Math (exact refactoring of the reference):
  mean_corr[b,j] = <qsum @ (wq @ wk.T), k[b,j]> / (H*L),  qsum = sum_i q[b,i]
  top7 delays d_k + softmax weights w_k over mean_corr
  out[b,l]      = (sum_k w_k * values[b,(l+d_k)%L]) @ (wv@wo)

Sharding: core c handles batch b=c//2, output half h=c%2 (rows [h*1024, ...)).
Host does layout/dtype-only prep (slice/transpose/fp16 cast); all FLOPs on
device.  All heavy matmuls run in fp16 (inputs quantized to fp16, fp32 psum
accumulation); verified numerically: top-7 delay sets match fp32 exactly on
the fixed inputs and overall rel err ~7e-4 (tolerance 2e-2).

Compute placement:
  PE : W1=wq@wkT, qsum, uT=W1^T.qsum, scores=uT.kT, W2=wv@wo, mix cols
       [0,640) via weighted-identity psum accumulation, all out matmuls
  ACT: psum->sbuf copies/downcasts, softmax exp (+accumulator sum), ext
       cols [0,512), mix scaled-copy stream for cols [640,896)
  DVE: topk (split max + max_index), small downcasts, weighted identities,
       adds for the ACT stream, stt chain for cols [896,1024), out staging;
       softmax 1/sum is deferred into the out stages
  Pool: circular vt extension cols [512,1024) (walrus rejects gpsimd
       stt/psum access, so Pool only copies)

Cost-model notes (TimelineSim v2): matmul cost = out-free-size x cycles/row
(f16=1); the PE clock ramps ~17us before full speed, so W1/qsum double as
ramp fodder and idle gaps are bridged with warmers; DMA is charged on one
exclusive aggregate device (~2.9us per fp16 MB); W1 must NOT be pre-scaled
by 1/(H*L) in fp16 (subnormals flip the fragile batch-3 top-7 margin) - the
scale lives in the softmax's scale/bias instead.
"""

import numpy as np
from contextlib import ExitStack

import concourse.bass as bass
import concourse.bacc as bacc
import concourse.mybir as mybir
import concourse.tile as tile
from concourse.bass_utils import run_bass_kernel_spmd

B, L, D, H = 4, 2048, 512, 8
HALF = L // 2          # 1024 output rows per core
KTOP = 7               # max(1, int(log(2048))) = 7
EXT = L + HALF         # values extended along L for wrap-free dynamic slicing
P = 128
FT = D // P            # 4 feature tiles
NT = L // P            # 16 sequence tiles
F32 = mybir.dt.float32
F16 = mybir.dt.float16
U32 = mybir.dt.uint32
I32 = mybir.dt.int32
AF = mybir.ActivationFunctionType
ALU = mybir.AluOpType
ENG = mybir.EngineType

# ---- mix column-range split (cols of the 1024 output rows) --------------
# strategy: PE scaled-identity psum accumulation / ACT mul + DVE add pipe /
#           DVE stt chain / Pool stt chain.  Ranges on 128 boundaries.
MIX_PE = (0, 640)      # psum tiles split 512+128 per fc
MIX_ACT = (640, 896)   # ACT mul stream + DVE adds
MIX_DVE = (896, 1024)  # DVE stt chain (ranges need not align to out chunks)
MIX_POOL = (1024, 1024)  # Pool cannot run scalar_tensor_tensor (walrus)


def _build():
    nc = bacc.Bacc()
    qp_d = nc.dram_tensor("qp", [P, NT, D], F16, kind="ExternalInput")
    ktp_d = nc.dram_tensor("ktp", [P, FT, L], F16, kind="ExternalInput")
    vt_d = nc.dram_tensor("vt", [P, FT, L], F16, kind="ExternalInput")
    wqk_d = nc.dram_tensor("wqk", [P, FT, 2, D], F16, kind="ExternalInput")
    wvo_d = nc.dram_tensor("wvo", [P, 2 * FT, D], F16, kind="ExternalInput")
    cst16_d = nc.dram_tensor("cst16", [P, P + 2], F16, kind="ExternalInput")
    cstr_d = nc.dram_tensor("cstr", [1, P + 8], F32, kind="ExternalInput")
    out_d = nc.dram_tensor("out", [HALF, D], F16, kind="ExternalOutput")

    with tile.TileContext(nc) as tc, ExitStack() as ctx:
        big = ctx.enter_context(tc.tile_pool(name="big", bufs=1))
        sm = ctx.enter_context(tc.tile_pool(name="sm", bufs=1))
        psA = ctx.enter_context(
            tc.tile_pool(name="psA", bufs=4, space=bass.MemorySpace.PSUM)
        )
        psB = ctx.enter_context(
            tc.tile_pool(name="psB", bufs=4, space=bass.MemorySpace.PSUM)
        )

        # ---- resident input packs; DMAs in priority order ---------------
        # wq/wk packs interleaved by contraction chunk: W1's first accs can
        # start as soon as the first half lands (earlier PE p-state ramp)
        wqk = big.tile([P, FT, 2, D], F16, tag="wqk")
        nc.sync.dma_start(wqk[:, 0:1], wqk_d[:, 0:1])
        nc.sync.dma_start(wqk[:, 1:2], wqk_d[:, 1:2])
        nc.sync.dma_start(wqk[:, 2:FT], wqk_d[:, 2:FT])

        cst16 = sm.tile([P, P + 2], F16, tag="cst16")
        nc.sync.dma_start(cst16[:], cst16_d[:])
        ident16 = cst16[:, 0:P]
        ones16 = cst16[:, P:P + 1]
        cstr = sm.tile([1, P + 8], F32, tag="cstr")
        nc.sync.dma_start(cstr[:], cstr_d[:])
        ones_row = cstr[0:1, 0:P]
        one1 = cstr[0:1, P:P + 1]

        qp = big.tile([P, NT, D], F16, tag="qp")
        nc.sync.dma_start(qp[:, 0:8, :], qp_d[:, 0:8, :])
        nc.sync.dma_start(qp[:, 8:NT, :], qp_d[:, 8:NT, :])

        ktp = big.tile([P, FT, L], F16, tag="ktp")
        nc.sync.dma_start(ktp[:, 0:2, :], ktp_d[:, 0:2, :])
        nc.sync.dma_start(ktp[:, 2:FT, :], ktp_d[:, 2:FT, :])

        wvo = big.tile([P, 2 * FT, D], F16, tag="wvo")
        nc.sync.dma_start(wvo[:], wvo_d[:])
        wvtp = wvo[:, 0:FT, :]
        wop = wvo[:, FT:2 * FT, :]

        vtE = big.tile([P, FT, EXT], F16, tag="vtE")
        nc.sync.dma_start(vtE[:, :, 0:HALF], vt_d[:, :, 0:HALF])
        nc.sync.dma_start(vtE[:, :, HALF:L], vt_d[:, :, HALF:L])

        # ---- small sbuf tiles -------------------------------------------
        w1_16 = big.tile([P, FT, D], F16, tag="w1")
        w2_16 = big.tile([P, FT, D], F16, tag="w2")
        aux = sm.tile([P, 10], F32, tag="aux")
        wbc = aux[:, 0:7]                # broadcast UNNORMALIZED exp weights
        se_bc = aux[:, 7:8]              # broadcast softmax denominator
        rse_bc = aux[:, 8:9]             # 1/denominator (per-partition)
        qsumT16 = sm.tile([P, 8], F16, tag="qsT")   # [:,0:4] qsumT, [:,4:8] uT
        uT16 = qsumT16[:, 4:8]
        srow = sm.tile([1, L + 64 + 2 * D], F32, tag="srow")
        qsum_sb = srow[0:1, L + 64:L + 64 + D]
        s_flat = srow[0:1, 0:L]
        vals8 = srow[0:1, L:L + 8]
        ex = srow[0:1, L + 8:L + 15]
        se = srow[0:1, L + 15:L + 16]    # adjacent to ex: [ex|se] broadcasts
        exse = srow[0:1, L + 8:L + 16]
        negm = srow[0:1, L + 16:L + 17]
        idx8 = srow[0:1, L + 32:L + 40].bitcast(U32)
        wI16 = sm.tile([P, KTOP * P], F16, tag="wI")
        acc16 = big.tile([P, FT, HALF], F16, tag="acc16")
        tk16 = big.tile([P, 2, FT, MIX_ACT[1] - MIX_ACT[0]], F16, tag="tk16")
        vmx16 = big.tile([P, FT, MIX_PE[1] - MIX_PE[0]], F16, tag="vmx16")

        # ---- W1 = wq @ wk.T (fp16), scaled by 1/(H*L) at downcast -------
        ps_w1 = [psA.tile([P, D], F32, tag="psa", bufs=4, name=f"ps_w1_{i}") for i in range(FT)]
        for mc in range(FT):
            for ic in range(FT):
                nc.tensor.matmul(
                    ps_w1[ic][:],
                    wqk[:, mc, 0, ic * P:(ic + 1) * P],
                    wqk[:, mc, 1, :],
                    start=(mc == 0),
                    stop=(mc == FT - 1),
                )
        # keep W1 at natural scale: scaling by 1/(H*L) here would push the
        # fp16 entries into subnormals (catastrophic rounding, flips topk);
        # the 1/(H*L) moves into the softmax scale/bias instead
        for ic in range(FT):
            nc.scalar.copy(w1_16[:, ic, :], ps_w1[ic][:])

        # ---- qsum = ones^T @ q  (psum f32) ------------------------------
        ps_qsum = psA.tile([1, D], F32, tag="psa", bufs=4, name="ps_qsum")
        for t in range(NT):
            nc.tensor.matmul(
                ps_qsum[:], ones16, qp[:, t, :],
                start=(t == 0), stop=(t == NT - 1),
            )
        nc.scalar.copy(qsum_sb, ps_qsum[:])

        # qsumT16 [128,4] via 4 tiny K=1 matmuls + DVE downcasts
        ps_qT = [psA.tile([P, 1], F32, tag="psa", bufs=4, name=f"ps_qT{c}") for c in range(FT)]
        for c in range(FT):
            nc.tensor.matmul(
                ps_qT[c][:], qsum_sb[0:1, c * P:(c + 1) * P], one1,
                start=True, stop=True,
            )
        for c in range(FT):
            nc.vector.tensor_copy(qsumT16[:, c:c + 1], ps_qT[c][:])

        # ---- uT directly: uT[j] = sum_c qsum[c] * W1[c,j] ---------------
        # (avoids the u row + transpose ping-pong: 16 tiny K-contraction
        # matmuls accumulate uT chunks straight into psum)
        ps_uT = [psA.tile([P, 1], F32, tag="psa", bufs=4, name=f"ps_uT{c}") for c in range(FT)]
        for cc in range(FT):
            for jc in range(FT):
                nc.tensor.matmul(
                    ps_uT[jc][:],
                    w1_16[:, cc, jc * P:(jc + 1) * P],
                    qsumT16[:, cc:cc + 1],
                    start=(cc == 0),
                    stop=(cc == FT - 1),
                )
        for c in range(FT):
            nc.vector.tensor_copy(uT16[:, c:c + 1], ps_uT[c][:])

        # ---- scores s[1,2048] = u . k_j  (4 psum banks of 512) ----------
        ps_s = [psB.tile([1, 512], F32, tag="psb", bufs=4, name=f"ps_s{j}") for j in range(FT)]
        # bank-major: bank j finishes after its 4 cc accs, so copies and the
        # max halves pipeline behind the still-running later banks
        for j in range(FT):
            for cc in range(FT):
                nc.tensor.matmul(
                    ps_s[j][:],
                    uT16[:, cc:cc + 1],
                    ktp[:, cc, j * 512:(j + 1) * 512],
                    start=(cc == 0),
                    stop=(cc == FT - 1),
                )

        # circular extension split across ACT and Pool idle windows so the
        # DVE chain (max/max_index -> regs -> wI) never carries it
        nc.scalar.copy(vtE[:, :, L:L + 512], vtE[:, :, 0:512])
        nc.gpsimd.tensor_copy(vtE[:, :, L + 512:EXT], vtE[:, :, 512:HALF])

        # ---- W2 = wv @ wo (fp16) on PE while DVE runs the topk ----------
        # psA ring (scores own psB); downcasts on ACT
        ps_w2 = [psA.tile([P, D], F32, tag="psa", bufs=4, name=f"ps_w2_{i}") for i in range(FT)]
        for mc in range(FT):
            for ic in range(FT):
                nc.tensor.matmul(
                    ps_w2[ic][:],
                    wvtp[:, mc, ic * P:(ic + 1) * P],
                    wop[:, mc, :],
                    start=(mc == 0),
                    stop=(mc == FT - 1),
                )
        # post-W2 warmers: bridge the PE gap across the topk so the p-state
        # stays at full clock; ps_out0 is reset by its start=True acc later
        ps_out0 = psA.tile([P, D], F32, tag="psa", bufs=4, name="ps_out0")
        for i in range(12):
            nc.tensor.matmul(ps_out0[:, 0:P], ident16, ident16,
                             start=True, stop=True)

        for j in range(FT):
            dst = s_flat[0:1, j * 512:(j + 1) * 512]
            if j == 1:
                nc.vector.tensor_copy(dst, ps_s[j][:])
            else:
                nc.scalar.copy(dst, ps_s[j][:])

        # ---- top-8 + softmax over first 7 -------------------------------
        # max in two pipelined halves (each starts as soon as its two score
        # banks are copied), merged by an 8+8 -> top8 pass
        v8a = srow[0:1, L + 40:L + 48]
        v8b = srow[0:1, L + 48:L + 56]
        nc.vector.max(v8a, s_flat[0:1, 0:1024])
        nc.vector.max(v8b, s_flat[0:1, 1024:2048])
        nc.vector.max(vals8, srow[0:1, L + 40:L + 56])
        nc.vector.tensor_scalar_mul(negm, vals8[0:1, 0:1], -1.0 / (H * L))
        nc.vector.max_index(idx8, vals8, s_flat)
        # Exp computes its own sum via the ACT accumulator (one op less on
        # the DVE, which is busy with max_index)
        nc.scalar.activation(
            ex, vals8[0:1, 0:KTOP], AF.Exp, bias=negm, scale=1.0 / (H * L),
            accum_out=se,
        )

        # broadcast [ex | sum] along partitions; normalization by 1/sum is
        # deferred to the final out-stage copies (linear), which keeps the
        # whole weight chain off the DVE while max_index runs
        ps_wbc = psB.tile([P, KTOP + 1], F32, tag="psb", bufs=4, name="ps_wbc")
        nc.tensor.matmul(ps_wbc[:], ones_row, exse, start=True, stop=True)
        nc.scalar.copy(aux[:, 0:8], ps_wbc[:])
        nc.vector.reciprocal(rse_bc, se_bc)


        # ---- delays into engine registers (right after topk; engine
        # queues are clean here so the cross-engine load doesn't stall) ----
        _, dks = nc.values_load_multi_w_load_instructions(
            idx8[0:1, 0:KTOP].bitcast(I32),
            engines=(ENG.PE, ENG.DVE, ENG.Activation),
            min_val=0,
            max_val=L - 1,
            skip_runtime_bounds_check=True,
        )

        # W2 downcasts AFTER the register loads: ACT's TensorLoad must not
        # queue behind them (the cross-engine load acts as a barrier)
        for ic in range(FT):
            nc.scalar.copy(w2_16[:, ic, :], ps_w2[ic][:])

        # weighted identities for the PE mix stream (DVE, JIT per k)
        for kk in range(KTOP):
            nc.vector.tensor_scalar_mul(
                wI16[:, kk * P:(kk + 1) * P], ident16, wbc[:, kk:kk + 1]
            )



        # ---- mix: engine streams over column ranges ---------------------
        # PE: psum accumulation with weighted identities, 512-col groups
        a0, a1 = MIX_PE
        groups = []
        g = a0
        while g < a1:
            gw = min(512, a1 - g)
            groups.append((g, gw))
            g += gw
        for gi, (gb, gw) in enumerate(groups):
            for fc in range(FT):
                ps_mx = psB.tile([P, gw], F32, tag="psb", bufs=4,
                                 name=f"ps_mx{gi}_{fc}")
                for kk in range(KTOP):
                    nc.tensor.matmul(
                        ps_mx[:],
                        wI16[:, kk * P:(kk + 1) * P],
                        vtE[:, fc, bass.ds(dks[kk] + gb, gw)],
                        start=(kk == 0),
                        stop=(kk == KTOP - 1),
                    )
                # downcast as soon as this chunk's accumulation finishes;
                # alternate ACT/DVE so neither serializes the PE stream
                dst = vmx16[:, fc, gb - a0:gb - a0 + gw]
                if fc % 2 == 0:
                    nc.scalar.copy(dst, ps_mx[:])
                else:
                    nc.vector.tensor_copy(dst, ps_mx[:])

        # ACT mul stream + DVE add chain
        b0, b1 = MIX_ACT
        nb = b1 - b0
        accA = acc16[:, :, b0:b1]
        nc.scalar.mul(accA[:], vtE[:, :, bass.ds(dks[0] + b0, nb)], wbc[:, 0:1])
        for kk in range(1, KTOP):
            tkb = tk16[:, kk % 2]
            nc.scalar.mul(
                tkb[:], vtE[:, :, bass.ds(dks[kk] + b0, nb)], wbc[:, kk:kk + 1]
            )
            nc.vector.tensor_tensor(accA[:], tkb[:], accA[:], ALU.add)

        # DVE stt chain (disabled when the range is empty)
        c0, c1 = MIX_DVE
        if c1 > c0:
            ncd = c1 - c0
            accD = acc16[:, :, c0:c1]
            nc.vector.tensor_scalar_mul(
                accD[:], vtE[:, :, bass.ds(dks[0] + c0, ncd)], wbc[:, 0:1]
            )
            for kk in range(1, KTOP):
                nc.vector.scalar_tensor_tensor(
                    accD[:],
                    vtE[:, :, bass.ds(dks[kk] + c0, ncd)],
                    wbc[:, kk:kk + 1],
                    accD[:],
                    ALU.mult,
                    ALU.add,
                )


        # ---- out rows: out[l,:] = sum_f vmixT[f,l] * W2[f,:] ------------
        # psum -> sbuf staging (DMA cannot read PSUM); copies alternate
        # ACT/DVE; one DMA per pair of chunks.
        NH = HALF // P
        # fp16 staging/output: halves the out DMA traffic; the host gather
        # widens to fp32 (value-preserving cast of the device's results)
        ostg = big.tile([P, NH, D], F16, tag="ostg")
        stage_eng = [1, 0, 1, 0, 1, 0, 1, 0]
        for lc in range(NH):
            lo = lc * P
            if lc == 0:
                ps_out = ps_out0
            else:
                ps_out = psA.tile([P, D], F32, tag="psa", bufs=4, name=f"ps_out{lc}")
            for ft in range(FT):
                if a0 <= lo < a1:
                    src = vmx16[:, ft, lo - a0:lo - a0 + P]
                else:
                    src = acc16[:, ft, lo:lo + P]
                nc.tensor.matmul(
                    ps_out[:], src, w2_16[:, ft, :],
                    start=(ft == 0), stop=(ft == FT - 1),
                )
            if stage_eng[lc] == 0:
                nc.scalar.mul(ostg[:, lc, :], ps_out[:], rse_bc)
            else:
                nc.vector.tensor_scalar_mul(ostg[:, lc, :], ps_out[:], rse_bc)
            nc.sync.dma_start(
                out_d.rearrange("(t p) c -> p t c", p=P)[:, lc:lc + 1, :],
                ostg[:, lc:lc + 1, :],
            )

    return nc


_NC = None
TRACE = False
_LAST_RESULTS = None


def _get_nc():
    global _NC
    if _NC is None:
        _NC = _build()
        _NC.finalize()
    return _NC


def _prep_consts():
    cst16 = np.zeros((P, P + 2), np.float16)
    cst16[:, 0:P] = np.eye(P, dtype=np.float16)
    cst16[:, P] = 1.0
    cstr = np.zeros((1, P + 8), np.float32)
    cstr[0, 0:P] = 1.0
    cstr[0, P] = 1.0
    return cst16, cstr


def kernel(queries, keys, values, wq, wk, wv, wo):
    nc = _get_nc()
    cst16, cstr = _prep_consts()
    f16 = np.float16

    def pack(m):
        # [512, 512] -> [128, 4, 512] with row index m = mc*128 + p
        return np.ascontiguousarray(
            m.reshape(FT, P, D).transpose(1, 0, 2).astype(f16)
        )

    wqk = np.ascontiguousarray(
        np.stack([pack(np.asarray(wq).T), pack(np.asarray(wk).T)], axis=2)
    )
    wvo = np.ascontiguousarray(
        np.concatenate([pack(np.asarray(wv).T), pack(np.asarray(wo))], axis=1)
    )

    in_maps = []
    for c in range(8):
        b, h = divmod(c, 2)
        qp = np.ascontiguousarray(
            queries[b].reshape(NT, P, D).transpose(1, 0, 2).astype(f16)
        )
        ktp = np.ascontiguousarray(
            keys[b].T.reshape(FT, P, L).transpose(1, 0, 2).astype(f16)
        )
        vrot = np.roll(values[b], -h * HALF, axis=0)
        vt = np.ascontiguousarray(
            vrot.T.reshape(FT, P, L).transpose(1, 0, 2).astype(f16)
        )
        in_maps.append(
            {
                "qp": qp,
                "ktp": ktp,
                "vt": vt,
                "wqk": wqk,
                "wvo": wvo,
                "cst16": cst16,
                "cstr": cstr,
            }
        )
    global _LAST_RESULTS
    res = run_bass_kernel_spmd(nc, in_maps, list(range(8)), trace=TRACE)
    _LAST_RESULTS = res
    out = np.empty((B, L, D), np.float32)
    for c in range(8):
        b, h = divmod(c, 2)
        out[b, h * HALF:(h + 1) * HALF] = res.results[c]["out"].astype(
            np.float32
        )
    return out


# revision 51
# speedup vs baseline: 1.1356x; 1.0034x over previous
"""AutoCorrelation (Autoformer-style) sparse attention kernel for 8 trn2 cores.

Math (exact refactoring of the reference):
  mean_corr[b,j] = <qsum @ (wq @ wk.T), k[b,j]> / (H*L),  qsum = sum_i q[b,i]
  top7 delays d_k + softmax weights w_k over mean_corr
  out[b,l]      = (sum_k w_k * values[b,(l+d_k)%L]) @ (wv@wo)

Sharding: core c handles batch b=c//2, output half h=c%2 (rows [h*1024, ...)).
Host does layout/dtype-only prep (slice/transpose/fp16 cast); all FLOPs on
device.  All heavy matmuls run in fp16 (inputs quantized to fp16, fp32 psum
accumulation); verified numerically: top-7 delay sets match fp32 exactly on
the fixed inputs and overall rel err ~7e-4 (tolerance 2e-2).

Compute placement:
  PE : W1=wq@wkT, qsum, uT=W1^T.qsum, scores=uT.kT, W2=wv@wo, mix cols
       [0,640) via weighted-identity psum accumulation, all out matmuls
  ACT: psum->sbuf copies/downcasts, softmax exp (+accumulator sum), ext
       cols [0,512), mix scaled-copy stream for cols [640,896)
  DVE: topk (split max + max_index), small downcasts, weighted identities,
       adds for the ACT stream, stt chain for cols [896,1024), out staging;
       softmax 1/sum is deferred into the out stages
  Pool: circular vt extension cols [512,1024) (walrus rejects gpsimd
       stt/psum access, so Pool only copies)

Cost-model notes (TimelineSim v2): matmul cost = out-free-size x cycles/row
(f16=1); the PE clock ramps ~17us before full speed, so W1/qsum double as
ramp fodder and idle gaps are bridged with warmers; DMA is charged on one
exclusive aggregate device (~2.9us per fp16 MB); W1 must NOT be pre-scaled
by 1/(H*L) in fp16 (subnormals flip the fragile batch-3 top-7 margin) - the
scale lives in the softmax's scale/bias instead.
"""

import numpy as np
from contextlib import ExitStack

import concourse.bass as bass
import concourse.bacc as bacc
import concourse.mybir as mybir
import concourse.tile as tile
from concourse.bass_utils import run_bass_kernel_spmd

B, L, D, H = 4, 2048, 512, 8
HALF = L // 2          # 1024 output rows per core
KTOP = 7               # max(1, int(log(2048))) = 7
EXT = L + HALF         # values extended along L for wrap-free dynamic slicing
P = 128
FT = D // P            # 4 feature tiles
NT = L // P            # 16 sequence tiles
F32 = mybir.dt.float32
F16 = mybir.dt.float16
U32 = mybir.dt.uint32
I32 = mybir.dt.int32
AF = mybir.ActivationFunctionType
ALU = mybir.AluOpType
ENG = mybir.EngineType

# ---- mix column-range split (cols of the 1024 output rows) --------------
# strategy: PE scaled-identity psum accumulation / ACT mul + DVE add pipe /
#           DVE stt chain / Pool stt chain.  Ranges on 128 boundaries.
MIX_PE = (0, 640)      # psum tiles split 512+128 per fc
MIX_ACT = (640, 888)   # ACT mul stream + DVE adds
MIX_DVE = (888, 1024)  # DVE stt chain (ranges need not align to out chunks)
MIX_POOL = (1024, 1024)  # Pool cannot run scalar_tensor_tensor (walrus)


def _build():
    nc = bacc.Bacc()
    qp_d = nc.dram_tensor("qp", [P, NT, D], F16, kind="ExternalInput")
    ktp_d = nc.dram_tensor("ktp", [P, FT, L], F16, kind="ExternalInput")
    vt_d = nc.dram_tensor("vt", [P, FT, L], F16, kind="ExternalInput")
    wqk_d = nc.dram_tensor("wqk", [P, FT, 2, D], F16, kind="ExternalInput")
    wvo_d = nc.dram_tensor("wvo", [P, 2 * FT, D], F16, kind="ExternalInput")
    cst16_d = nc.dram_tensor("cst16", [P, P + 2], F16, kind="ExternalInput")
    cstr_d = nc.dram_tensor("cstr", [1, P + 8], F32, kind="ExternalInput")
    out_d = nc.dram_tensor("out", [HALF, D], F16, kind="ExternalOutput")

    with tile.TileContext(nc) as tc, ExitStack() as ctx:
        big = ctx.enter_context(tc.tile_pool(name="big", bufs=1))
        sm = ctx.enter_context(tc.tile_pool(name="sm", bufs=1))
        psA = ctx.enter_context(
            tc.tile_pool(name="psA", bufs=4, space=bass.MemorySpace.PSUM)
        )
        psB = ctx.enter_context(
            tc.tile_pool(name="psB", bufs=4, space=bass.MemorySpace.PSUM)
        )

        # ---- resident input packs; DMAs in priority order ---------------
        # wq/wk packs interleaved by contraction chunk: W1's first accs can
        # start as soon as the first half lands (earlier PE p-state ramp)
        wqk = big.tile([P, FT, 2, D], F16, tag="wqk")
        nc.sync.dma_start(wqk[:, 0:1], wqk_d[:, 0:1])
        nc.sync.dma_start(wqk[:, 1:2], wqk_d[:, 1:2])
        nc.sync.dma_start(wqk[:, 2:FT], wqk_d[:, 2:FT])

        cst16 = sm.tile([P, P + 2], F16, tag="cst16")
        nc.sync.dma_start(cst16[:], cst16_d[:])
        ident16 = cst16[:, 0:P]
        ones16 = cst16[:, P:P + 1]
        cstr = sm.tile([1, P + 8], F32, tag="cstr")
        nc.sync.dma_start(cstr[:], cstr_d[:])
        ones_row = cstr[0:1, 0:P]
        one1 = cstr[0:1, P:P + 1]

        qp = big.tile([P, NT, D], F16, tag="qp")
        nc.sync.dma_start(qp[:, 0:8, :], qp_d[:, 0:8, :])
        nc.sync.dma_start(qp[:, 8:NT, :], qp_d[:, 8:NT, :])

        ktp = big.tile([P, FT, L], F16, tag="ktp")
        nc.sync.dma_start(ktp[:, 0:2, :], ktp_d[:, 0:2, :])
        nc.sync.dma_start(ktp[:, 2:FT, :], ktp_d[:, 2:FT, :])

        wvo = big.tile([P, 2 * FT, D], F16, tag="wvo")
        nc.sync.dma_start(wvo[:], wvo_d[:])
        wvtp = wvo[:, 0:FT, :]
        wop = wvo[:, FT:2 * FT, :]

        vtE = big.tile([P, FT, EXT], F16, tag="vtE")
        nc.sync.dma_start(vtE[:, :, 0:HALF], vt_d[:, :, 0:HALF])
        nc.sync.dma_start(vtE[:, :, HALF:L], vt_d[:, :, HALF:L])

        # ---- small sbuf tiles -------------------------------------------
        w1_16 = big.tile([P, FT, D], F16, tag="w1")
        w2_16 = big.tile([P, FT, D], F16, tag="w2")
        aux = sm.tile([P, 10], F32, tag="aux")
        wbc = aux[:, 0:7]                # broadcast UNNORMALIZED exp weights
        se_bc = aux[:, 7:8]              # broadcast softmax denominator
        rse_bc = aux[:, 8:9]             # 1/denominator (per-partition)
        qsumT16 = sm.tile([P, 8], F16, tag="qsT")   # [:,0:4] qsumT, [:,4:8] uT
        uT16 = qsumT16[:, 4:8]
        srow = sm.tile([1, L + 64 + 2 * D], F32, tag="srow")
        qsum_sb = srow[0:1, L + 64:L + 64 + D]
        s_flat = srow[0:1, 0:L]
        vals8 = srow[0:1, L:L + 8]
        ex = srow[0:1, L + 8:L + 15]
        se = srow[0:1, L + 15:L + 16]    # adjacent to ex: [ex|se] broadcasts
        exse = srow[0:1, L + 8:L + 16]
        negm = srow[0:1, L + 16:L + 17]
        idx8 = srow[0:1, L + 32:L + 40].bitcast(U32)
        wI16 = sm.tile([P, KTOP * P], F16, tag="wI")
        acc16 = big.tile([P, FT, HALF], F16, tag="acc16")
        tk16 = big.tile([P, 2, FT, MIX_ACT[1] - MIX_ACT[0]], F16, tag="tk16")
        vmx16 = big.tile([P, FT, MIX_PE[1] - MIX_PE[0]], F16, tag="vmx16")

        # ---- W1 = wq @ wk.T (fp16), scaled by 1/(H*L) at downcast -------
        ps_w1 = [psA.tile([P, D], F32, tag="psa", bufs=4, name=f"ps_w1_{i}") for i in range(FT)]
        for mc in range(FT):
            for ic in range(FT):
                nc.tensor.matmul(
                    ps_w1[ic][:],
                    wqk[:, mc, 0, ic * P:(ic + 1) * P],
                    wqk[:, mc, 1, :],
                    start=(mc == 0),
                    stop=(mc == FT - 1),
                )
        # keep W1 at natural scale: scaling by 1/(H*L) here would push the
        # fp16 entries into subnormals (catastrophic rounding, flips topk);
        # the 1/(H*L) moves into the softmax scale/bias instead
        for ic in range(FT):
            nc.scalar.copy(w1_16[:, ic, :], ps_w1[ic][:])

        # ---- qsum = ones^T @ q  (psum f32) ------------------------------
        ps_qsum = psA.tile([1, D], F32, tag="psa", bufs=4, name="ps_qsum")
        for t in range(NT):
            nc.tensor.matmul(
                ps_qsum[:], ones16, qp[:, t, :],
                start=(t == 0), stop=(t == NT - 1),
            )
        nc.scalar.copy(qsum_sb, ps_qsum[:])

        # qsumT16 [128,4] via 4 tiny K=1 matmuls + DVE downcasts
        ps_qT = [psA.tile([P, 1], F32, tag="psa", bufs=4, name=f"ps_qT{c}") for c in range(FT)]
        for c in range(FT):
            nc.tensor.matmul(
                ps_qT[c][:], qsum_sb[0:1, c * P:(c + 1) * P], one1,
                start=True, stop=True,
            )
        for c in range(FT):
            nc.vector.tensor_copy(qsumT16[:, c:c + 1], ps_qT[c][:])

        # ---- uT directly: uT[j] = sum_c qsum[c] * W1[c,j] ---------------
        # (avoids the u row + transpose ping-pong: 16 tiny K-contraction
        # matmuls accumulate uT chunks straight into psum)
        ps_uT = [psA.tile([P, 1], F32, tag="psa", bufs=4, name=f"ps_uT{c}") for c in range(FT)]
        for cc in range(FT):
            for jc in range(FT):
                nc.tensor.matmul(
                    ps_uT[jc][:],
                    w1_16[:, cc, jc * P:(jc + 1) * P],
                    qsumT16[:, cc:cc + 1],
                    start=(cc == 0),
                    stop=(cc == FT - 1),
                )
        for c in range(FT):
            nc.vector.tensor_copy(uT16[:, c:c + 1], ps_uT[c][:])

        # ---- scores s[1,2048] = u . k_j  (4 psum banks of 512) ----------
        ps_s = [psB.tile([1, 512], F32, tag="psb", bufs=4, name=f"ps_s{j}") for j in range(FT)]
        # bank-major: bank j finishes after its 4 cc accs, so copies and the
        # max halves pipeline behind the still-running later banks
        for j in range(FT):
            for cc in range(FT):
                nc.tensor.matmul(
                    ps_s[j][:],
                    uT16[:, cc:cc + 1],
                    ktp[:, cc, j * 512:(j + 1) * 512],
                    start=(cc == 0),
                    stop=(cc == FT - 1),
                )

        # circular extension split across ACT and Pool idle windows so the
        # DVE chain (max/max_index -> regs -> wI) never carries it
        nc.scalar.copy(vtE[:, :, L:L + 512], vtE[:, :, 0:512])
        nc.gpsimd.tensor_copy(vtE[:, :, L + 512:EXT], vtE[:, :, 512:HALF])

        # ---- W2 = wv @ wo (fp16) on PE while DVE runs the topk ----------
        # psA ring (scores own psB); downcasts on ACT
        ps_w2 = [psA.tile([P, D], F32, tag="psa", bufs=4, name=f"ps_w2_{i}") for i in range(FT)]
        for mc in range(FT):
            for ic in range(FT):
                nc.tensor.matmul(
                    ps_w2[ic][:],
                    wvtp[:, mc, ic * P:(ic + 1) * P],
                    wop[:, mc, :],
                    start=(mc == 0),
                    stop=(mc == FT - 1),
                )
        # post-W2 warmers: bridge the PE gap across the topk so the p-state
        # stays at full clock; ps_out0 is reset by its start=True acc later
        ps_out0 = psA.tile([P, D], F32, tag="psa", bufs=4, name="ps_out0")
        for i in range(12):
            nc.tensor.matmul(ps_out0[:, 0:P], ident16, ident16,
                             start=True, stop=True)

        for j in range(FT):
            dst = s_flat[0:1, j * 512:(j + 1) * 512]
            if j == 1:
                nc.vector.tensor_copy(dst, ps_s[j][:])
            else:
                nc.scalar.copy(dst, ps_s[j][:])

        # ---- top-8 + softmax over first 7 -------------------------------
        # max in two pipelined halves (each starts as soon as its two score
        # banks are copied), merged by an 8+8 -> top8 pass
        v8a = srow[0:1, L + 40:L + 48]
        v8b = srow[0:1, L + 48:L + 56]
        nc.vector.max(v8a, s_flat[0:1, 0:1024])
        nc.vector.max(v8b, s_flat[0:1, 1024:2048])
        nc.vector.max(vals8, srow[0:1, L + 40:L + 56])
        nc.vector.tensor_scalar_mul(negm, vals8[0:1, 0:1], -1.0 / (H * L))
        nc.vector.max_index(idx8, vals8, s_flat)
        # Exp computes its own sum via the ACT accumulator (one op less on
        # the DVE, which is busy with max_index)
        nc.scalar.activation(
            ex, vals8[0:1, 0:KTOP], AF.Exp, bias=negm, scale=1.0 / (H * L),
            accum_out=se,
        )

        # broadcast [ex | sum] along partitions; normalization by 1/sum is
        # deferred to the final out-stage copies (linear), which keeps the
        # whole weight chain off the DVE while max_index runs
        ps_wbc = psB.tile([P, KTOP + 1], F32, tag="psb", bufs=4, name="ps_wbc")
        nc.tensor.matmul(ps_wbc[:], ones_row, exse, start=True, stop=True)
        nc.scalar.copy(aux[:, 0:8], ps_wbc[:])
        nc.vector.reciprocal(rse_bc, se_bc)


        # ---- delays into engine registers (right after topk; engine
        # queues are clean here so the cross-engine load doesn't stall) ----
        _, dks = nc.values_load_multi_w_load_instructions(
            idx8[0:1, 0:KTOP].bitcast(I32),
            engines=(ENG.PE, ENG.DVE, ENG.Activation),
            min_val=0,
            max_val=L - 1,
            skip_runtime_bounds_check=True,
        )

        # W2 downcasts AFTER the register loads: ACT's TensorLoad must not
        # queue behind them (the cross-engine load acts as a barrier)
        for ic in range(FT):
            nc.scalar.copy(w2_16[:, ic, :], ps_w2[ic][:])

        # weighted identities for the PE mix stream (DVE, JIT per k)
        for kk in range(KTOP):
            nc.vector.tensor_scalar_mul(
                wI16[:, kk * P:(kk + 1) * P], ident16, wbc[:, kk:kk + 1]
            )



        # ---- mix: engine streams over column ranges ---------------------
        # PE: psum accumulation with weighted identities, 512-col groups
        a0, a1 = MIX_PE
        groups = []
        g = a0
        while g < a1:
            gw = min(512, a1 - g)
            groups.append((g, gw))
            g += gw
        for gi, (gb, gw) in enumerate(groups):
            for fc in range(FT):
                ps_mx = psB.tile([P, gw], F32, tag="psb", bufs=4,
                                 name=f"ps_mx{gi}_{fc}")
                for kk in range(KTOP):
                    nc.tensor.matmul(
                        ps_mx[:],
                        wI16[:, kk * P:(kk + 1) * P],
                        vtE[:, fc, bass.ds(dks[kk] + gb, gw)],
                        start=(kk == 0),
                        stop=(kk == KTOP - 1),
                    )
                # downcast as soon as this chunk's accumulation finishes;
                # alternate ACT/DVE so neither serializes the PE stream
                dst = vmx16[:, fc, gb - a0:gb - a0 + gw]
                if fc % 2 == 0:
                    nc.scalar.copy(dst, ps_mx[:])
                else:
                    nc.vector.tensor_copy(dst, ps_mx[:])

        # ACT mul stream + DVE add chain
        b0, b1 = MIX_ACT
        nb = b1 - b0
        accA = acc16[:, :, b0:b1]
        nc.scalar.mul(accA[:], vtE[:, :, bass.ds(dks[0] + b0, nb)], wbc[:, 0:1])
        for kk in range(1, KTOP):
            tkb = tk16[:, kk % 2]
            nc.scalar.mul(
                tkb[:], vtE[:, :, bass.ds(dks[kk] + b0, nb)], wbc[:, kk:kk + 1]
            )
            nc.vector.tensor_tensor(accA[:], tkb[:], accA[:], ALU.add)

        # DVE stt chain (disabled when the range is empty)
        c0, c1 = MIX_DVE
        if c1 > c0:
            ncd = c1 - c0
            accD = acc16[:, :, c0:c1]
            nc.vector.tensor_scalar_mul(
                accD[:], vtE[:, :, bass.ds(dks[0] + c0, ncd)], wbc[:, 0:1]
            )
            for kk in range(1, KTOP):
                nc.vector.scalar_tensor_tensor(
                    accD[:],
                    vtE[:, :, bass.ds(dks[kk] + c0, ncd)],
                    wbc[:, kk:kk + 1],
                    accD[:],
                    ALU.mult,
                    ALU.add,
                )


        # ---- out rows: out[l,:] = sum_f vmixT[f,l] * W2[f,:] ------------
        # psum -> sbuf staging (DMA cannot read PSUM); copies alternate
        # ACT/DVE; one DMA per pair of chunks.
        NH = HALF // P
        # fp16 staging/output: halves the out DMA traffic; the host gather
        # widens to fp32 (value-preserving cast of the device's results)
        ostg = big.tile([P, NH, D], F16, tag="ostg")
        stage_eng = [1, 0, 1, 0, 1, 0, 1, 0]
        for lc in range(NH):
            lo = lc * P
            if lc == 0:
                ps_out = ps_out0
            else:
                ps_out = psA.tile([P, D], F32, tag="psa", bufs=4, name=f"ps_out{lc}")
            for ft in range(FT):
                if a0 <= lo < a1:
                    src = vmx16[:, ft, lo - a0:lo - a0 + P]
                else:
                    src = acc16[:, ft, lo:lo + P]
                nc.tensor.matmul(
                    ps_out[:], src, w2_16[:, ft, :],
                    start=(ft == 0), stop=(ft == FT - 1),
                )
            if stage_eng[lc] == 0:
                nc.scalar.mul(ostg[:, lc, :], ps_out[:], rse_bc)
            else:
                nc.vector.tensor_scalar_mul(ostg[:, lc, :], ps_out[:], rse_bc)
            nc.sync.dma_start(
                out_d.rearrange("(t p) c -> p t c", p=P)[:, lc:lc + 1, :],
                ostg[:, lc:lc + 1, :],
            )

    return nc


_NC = None
TRACE = False
_LAST_RESULTS = None


def _get_nc():
    global _NC
    if _NC is None:
        _NC = _build()
        _NC.finalize()
    return _NC


def _prep_consts():
    cst16 = np.zeros((P, P + 2), np.float16)
    cst16[:, 0:P] = np.eye(P, dtype=np.float16)
    cst16[:, P] = 1.0
    cstr = np.zeros((1, P + 8), np.float32)
    cstr[0, 0:P] = 1.0
    cstr[0, P] = 1.0
    return cst16, cstr


def kernel(queries, keys, values, wq, wk, wv, wo):
    nc = _get_nc()
    cst16, cstr = _prep_consts()
    f16 = np.float16

    def pack(m):
        # [512, 512] -> [128, 4, 512] with row index m = mc*128 + p
        return np.ascontiguousarray(
            m.reshape(FT, P, D).transpose(1, 0, 2).astype(f16)
        )

    wqk = np.ascontiguousarray(
        np.stack([pack(np.asarray(wq).T), pack(np.asarray(wk).T)], axis=2)
    )
    wvo = np.ascontiguousarray(
        np.concatenate([pack(np.asarray(wv).T), pack(np.asarray(wo))], axis=1)
    )

    in_maps = []
    for c in range(8):
        b, h = divmod(c, 2)
        qp = np.ascontiguousarray(
            queries[b].reshape(NT, P, D).transpose(1, 0, 2).astype(f16)
        )
        ktp = np.ascontiguousarray(
            keys[b].T.reshape(FT, P, L).transpose(1, 0, 2).astype(f16)
        )
        vrot = np.roll(values[b], -h * HALF, axis=0)
        vt = np.ascontiguousarray(
            vrot.T.reshape(FT, P, L).transpose(1, 0, 2).astype(f16)
        )
        in_maps.append(
            {
                "qp": qp,
                "ktp": ktp,
                "vt": vt,
                "wqk": wqk,
                "wvo": wvo,
                "cst16": cst16,
                "cstr": cstr,
            }
        )
    global _LAST_RESULTS
    res = run_bass_kernel_spmd(nc, in_maps, list(range(8)), trace=TRACE)
    _LAST_RESULTS = res
    out = np.empty((B, L, D), np.float32)
    for c in range(8):
        b, h = divmod(c, 2)
        out[b, h * HALF:(h + 1) * HALF] = res.results[c]["out"].astype(
            np.float32
        )
    return out
